# revision 6
# baseline (speedup 1.0000x reference)
"""Multi-head 2D self-attention (B=16, C_in=256, C_out=512, 8 heads, 32x32)
as a TRN2 Bass/Tile kernel.

Sharding: pure data-parallel over batch B=16 across the 8 NeuronCores
(2 batch elements per core, no collectives). Heads stay on-core.

Per-core algorithm (per batch element, M = 32*32 = 1024 tokens):
  q = Wq @ x + r ; k = Wk @ x + r        layout (c_out on partitions, M free)
  vT = x.T @ Wv.T                        layout (tokens on partitions, c_out
                                          free), stored bf16 with a 64-wide
                                          ones block per head
  per "group" (head h = 2*hp + off/64; 8 groups per batch element):
    ST[n, m] = sum_d k[d, n] * q[d, m]   (PE; keys on partitions so softmax
                                          needs no transpose before A @ V)
    E = exp(ST / 8)                      (ACT, bf16 out. |logits| < 14 so the
                                          reference's clip(+-50) never fires
                                          and exp stays in fp32 range.)
    out'[d, m], s[m] = sum_n vTe[n, [v|1]] * E[n, m]
                                         (PE; the ones block lands the softmax
                                          denominator s on partitions 64..127)
    out = out' * (1/s)                   (DVE reciprocal + multiply)

The attention groups are SOFTWARE-PIPELINED: in program order, the QK
matmuls + exp of group g+1 are interleaved (at key-tile granularity) with
the AV matmuls of group g. Engines have in-order queues, so without this
the TensorE/ScalarE/VectorE stages ping-pong serially (measured ~310us per
iteration steady-state); with it ScalarE (the exp payload, ~145us) paces
the whole kernel.

Matmul operands for the projections and QK are float32r (TF32-like PE mode,
1 cycle/row). E and vTe are bf16: halves SBUF pressure, same PE speed, and
the softmax normalization cancels most of the quantization error (measured
rel err ~2e-3 vs the fp32 reference, tolerance 2e-2).
"""

import os

import numpy as np

B_TOTAL, C_IN, C_OUT, HEADS = 16, 256, 512, 8
H_IMG = W_IMG = 32
M = H_IMG * W_IMG            # 1024 tokens
DH = C_OUT // HEADS          # 64
N_CORES = 8
B_LOC = B_TOTAL // N_CORES   # 2
KT = C_IN // 128             # 2 contraction tiles for the projections
CT = C_OUT // 128            # 4 c_out tiles == head pairs
MT = M // 128                # 8 token tiles
VE = 2 * DH                  # 128: head channels + 64 ones columns
NG = 2 * CT                  # 8 attention groups (one per head)


def _pe2d() -> np.ndarray:
    """Sinusoidal 2D positional encoding, (C_OUT, M) float32 (matches the
    reference's _pe2d)."""
    c, h, w = C_OUT, H_IMG, W_IMG
    d = c // 2

    def pe1d(dd, ll):
        pos = np.arange(ll, dtype=np.float32)[:, None]
        div = np.exp(
            -np.log(np.float32(10000.0))
            * np.arange(0, dd, 2, dtype=np.float32)
            / np.float32(dd)
        ).astype(np.float32)
        ang = (pos * div).astype(np.float32)
        pe = np.zeros((ll, dd), dtype=np.float32)
        pe[:, 0::2] = np.sin(ang)
        pe[:, 1::2] = np.cos(ang)
        return pe

    pe_y = np.broadcast_to(pe1d(d, h)[:, None, :], (h, w, d))
    pe_x = np.broadcast_to(pe1d(d, w)[None, :, :], (h, w, d))
    pe = np.concatenate([pe_y, pe_x], axis=-1)
    return np.ascontiguousarray(pe.reshape(h * w, c).T.astype(np.float32))


_BUILT = {}
LAST_RESULT = None


def _build(mode: str, repeats: int = 1):
    """Build (once) the Bass module for one core. Returns nc.

    repeats>1 re-emits the whole compute body N times (same inputs/outputs)
    — only used for timing: the time-vs-repeats slope isolates device time
    from the fixed axon dispatch overhead."""
    key = (mode, repeats)
    if key in _BUILT:
        return _BUILT[key]

    from contextlib import ExitStack

    import concourse.bass as bass
    import concourse.mybir as mybir
    import concourse.tile as tile
    from concourse import bacc

    f32 = mybir.dt.float32
    bf16 = mybir.dt.bfloat16
    if mode == "bf16":
        st_dt = mybir.dt.bfloat16
    elif mode == "f32r":
        # TF32-like PE mode: 1 cycle/row (float32 is 4); same 4-byte storage.
        st_dt = mybir.dt.float32r
    else:
        st_dt = f32

    nc = bacc.Bacc("TRN2", num_devices=N_CORES)

    x_d = nc.dram_tensor("x", (B_LOC, C_IN, M), st_dt, kind="ExternalInput").ap()
    wq_d = nc.dram_tensor("wqT", (C_IN, C_OUT), st_dt, kind="ExternalInput").ap()
    wk_d = nc.dram_tensor("wkT", (C_IN, C_OUT), st_dt, kind="ExternalInput").ap()
    wv_d = nc.dram_tensor("wvT", (C_IN, C_OUT), st_dt, kind="ExternalInput").ap()
    r_d = nc.dram_tensor("r", (C_OUT, M), f32, kind="ExternalInput").ap()
    ones_d = nc.dram_tensor("ones", (1, 512), bf16, kind="ExternalInput").ap()
    out_d = nc.dram_tensor("out", (B_LOC, C_OUT, M), f32, kind="ExternalOutput").ap()

    EXP = mybir.ActivationFunctionType.Exp

    with tile.TileContext(nc) as tc:
        with ExitStack() as ctx:
            consts = ctx.enter_context(tc.tile_pool(name="consts", bufs=1))
            xpool = ctx.enter_context(tc.tile_pool(name="xpool", bufs=1))
            qkpool = ctx.enter_context(tc.tile_pool(name="qkpool", bufs=1))
            vpool = ctx.enter_context(tc.tile_pool(name="vpool", bufs=2))
            epool = ctx.enter_context(tc.tile_pool(name="epool", bufs=18))
            opool = ctx.enter_context(tc.tile_pool(name="opool", bufs=4))
            rcpool = ctx.enter_context(tc.tile_pool(name="rcpool", bufs=4))
            mmpool = ctx.enter_context(tc.tile_pool(name="mmpool", bufs=3, space="PSUM"))
            accpool = ctx.enter_context(tc.tile_pool(name="accpool", bufs=2, space="PSUM"))

            # ---- constants: weights (transposed on host) and pos-encoding
            wt = {}
            for name, dram in (("q", wq_d), ("k", wk_d), ("v", wv_d)):
                for kt in range(KT):
                    t = consts.tile([128, C_OUT], st_dt, tag=f"w{name}{kt}")
                    nc.sync.dma_start(t[:], dram[kt * 128 : (kt + 1) * 128, :])
                    wt[name, kt] = t
            r_t = []
            for ct in range(CT):
                t = consts.tile([128, M], f32, tag=f"r{ct}")
                nc.sync.dma_start(t[:], r_d[ct * 128 : (ct + 1) * 128, :])
                r_t.append(t)

            # ---- x: all batches up front
            x_t = {}
            for b in range(B_LOC):
                for kt in range(KT):
                    t = xpool.tile([128, M], st_dt, tag=f"x{b}_{kt}")
                    nc.sync.dma_start(t[:], x_d[b, kt * 128 : (kt + 1) * 128, :])
                    x_t[b, kt] = t

            for _rep in range(repeats):
              for b in range(B_LOC):
                # ---- projections: q, k in (c_out, M) layout, + r
                q_t, k_t = [], []
                for name, dst in (("q", q_t), ("k", k_t)):
                    for ct in range(CT):
                        ps = mmpool.tile([128, M], f32, tag="mm")
                        for kt in range(KT):
                            for nh in range(2):
                                nc.tensor.matmul(
                                    ps[:, nh * 512 : (nh + 1) * 512],
                                    wt[name, kt][:, ct * 128 : (ct + 1) * 128],
                                    x_t[b, kt][:, nh * 512 : (nh + 1) * 512],
                                    start=(kt == 0),
                                    stop=(kt == KT - 1),
                                )
                        sb = qkpool.tile([128, M], st_dt, tag=f"{name}{ct}")
                        nc.vector.tensor_add(sb[:], ps[:], r_t[ct][:])
                        dst.append(sb)

                # ---- v in transposed (tokens, c_out) layout, with ones cols
                vte = []
                for mt in range(MT):
                    ps = accpool.tile([128, 512], f32, tag="acc")
                    for kt in range(KT):
                        nc.tensor.matmul(
                            ps[:],
                            x_t[b, kt][:, mt * 128 : (mt + 1) * 128],
                            wt["v", kt][:],
                            start=(kt == 0),
                            stop=(kt == KT - 1),
                        )
                    vt = vpool.tile([128, HEADS * VE], bf16, tag=f"v{mt}")
                    v3 = vt[:].rearrange("p (h e) -> p h e", e=VE)
                    nc.vector.tensor_copy(
                        v3[:, :, 0:DH], ps[:].rearrange("p (h e) -> p h e", e=DH)
                    )
                    # ones block via DMA broadcast from DRAM
                    nc.sync.dma_start(
                        v3[:, :, DH:VE],
                        bass.AP(
                            tensor=ones_d.tensor,
                            offset=ones_d.offset,
                            ap=[[0, 128], [1, HEADS * DH]],
                        ),
                    )
                    vte.append(vt)

                # ---- attention: 8 groups (one head each), software-pipelined
                # so QK+exp of group g interleaves with AV of group g-1.
                es = {}
                for g in range(NG + 1):
                    hp, off = g >> 1, 64 * (g & 1)
                    if g > 0:
                        ph = ((g - 1) >> 1) * 2 + ((g - 1) & 1)  # prev head
                        acc0 = accpool.tile([128, 512], f32, tag="acc")
                        acc1 = accpool.tile([128, 512], f32, tag="acc")
                        acc = [acc0, acc1]
                    for nt in range(MT):
                        if g < NG:
                            ps = mmpool.tile([128, M], f32, tag="mm")
                            for mh in range(2):
                                nc.tensor.matmul(
                                    ps[:, mh * 512 : (mh + 1) * 512],
                                    k_t[hp][off : off + 64, nt * 128 : (nt + 1) * 128],
                                    q_t[hp][off : off + 64, mh * 512 : (mh + 1) * 512],
                                    start=True,
                                    stop=True,
                                )
                            e = epool.tile([128, M], bf16, tag="e")
                            nc.scalar.activation(e[:], ps[:], EXP, scale=0.125)
                            es[g, nt] = e
                        if g > 0:
                            ep = es[g - 1, nt]
                            for mh in range(2):
                                nc.tensor.matmul(
                                    acc[mh][0:VE, :],
                                    vte[nt][:, ph * VE : (ph + 1) * VE],
                                    ep[:, mh * 512 : (mh + 1) * 512],
                                    start=(nt == 0),
                                    stop=(nt == MT - 1),
                                )
                    if g > 0:
                        o = opool.tile([DH, M], f32, tag="o")
                        for mh in range(2):
                            # acc rows 64..127 all hold s = sum_n E
                            rr = rcpool.tile([DH, 512], f32, tag="rc")
                            nc.vector.reciprocal(rr[:], acc[mh][DH:VE, :])
                            nc.vector.tensor_mul(
                                o[:, mh * 512 : (mh + 1) * 512],
                                acc[mh][0:DH, :],
                                rr[:],
                            )
                        nc.sync.dma_start(
                            out_d[b, ph * DH : (ph + 1) * DH, :], o[:]
                        )

    nc.compile()
    _BUILT[key] = nc
    return nc


def _prep_in_maps(x, Wq, Wk, Wv, mode: str):
    import ml_dtypes

    cast_dt = ml_dtypes.bfloat16 if mode == "bf16" else np.float32
    xf = np.ascontiguousarray(x.reshape(B_TOTAL, C_IN, M)).astype(cast_dt)
    wqT = np.ascontiguousarray(np.asarray(Wq, dtype=np.float32).T).astype(cast_dt)
    wkT = np.ascontiguousarray(np.asarray(Wk, dtype=np.float32).T).astype(cast_dt)
    wvT = np.ascontiguousarray(np.asarray(Wv, dtype=np.float32).T).astype(cast_dt)
    r = _pe2d()
    ones = np.ones((1, 512), dtype=ml_dtypes.bfloat16)
    in_maps = []
    for c in range(N_CORES):
        in_maps.append(
            {
                "x": np.ascontiguousarray(xf[c * B_LOC : (c + 1) * B_LOC]),
                "wqT": wqT,
                "wkT": wkT,
                "wvT": wvT,
                "r": r,
                "ones": ones,
            }
        )
    return in_maps


def kernel(x, Wq, Wk, Wv):
    mode = os.environ.get("ATTN_MM_MODE", "f32r")
    x = np.asarray(x, dtype=np.float32)
    nc = _build(mode)
    in_maps = _prep_in_maps(x, Wq, Wk, Wv, mode)

    from concourse import bass_utils

    res = bass_utils.run_bass_kernel_spmd(
        nc, in_maps, core_ids=list(range(N_CORES))
    )
    global LAST_RESULT
    LAST_RESULT = res
    out = np.concatenate([res.results[c]["out"] for c in range(N_CORES)], axis=0)
    return np.ascontiguousarray(
        out.reshape(B_TOTAL, C_OUT, H_IMG, W_IMG).astype(np.float32)
    )


if __name__ == "__main__":
    rng = np.random.default_rng(0)
    x = rng.standard_normal((B_TOTAL, C_IN, H_IMG, W_IMG), dtype=np.float32)
    s = 1.0 / np.sqrt(C_IN)
    Wq = rng.standard_normal((C_OUT, C_IN), dtype=np.float32) * s
    Wk = rng.standard_normal((C_OUT, C_IN), dtype=np.float32) * s
    Wv = rng.standard_normal((C_OUT, C_IN), dtype=np.float32) * s
    out = kernel(x, Wq, Wk, Wv)
    print(out.shape, out.dtype, float(np.abs(out).max()))
